# revision 2
# baseline (speedup 1.0000x reference)
"""Trainium2 Bass kernel for nn_AF_2 (dense per-branch MLP gating).

Math (reference):
    s = t.sum(axis=1)                                  # (B, D)
    h = relu(BN1(einsum('nid,bd->bni', W1, s) + b1))   # (B, NB, I)
    y = BN2(einsum('ndi,bni->bnd', W2, h) + b2)        # (B, NB, D)
    out = (sigmoid(y) * t).sum(axis=1) * 3             # (B, D)

Strategy (v2):
  - 8-way data parallel over B (512 rows/core), zero collectives.
  - Host folds the (inference-mode) BatchNorms into W/b and packs ALL
    device inputs into two DRAM params (one bf16 blob: t | W1 | W2 | ident,
    one f32 blob: b1 | b2) to minimize per-execution argument overhead.
  - t is streamed twice (pass A: s accumulation, pass B: gating product)
    in 1MB chunks, alternating between BOTH HWDGE queues (SP + Act) in
    pass A; pass B chunks ride the SP queue while weights trickle in on
    the Pool SWDGE queue, paced by the branch pipeline.
  - s^T accumulated with TensorE identity-matmuls into PSUM (exact f32).
  - GEMM1/GEMM2 are per-branch bf16 matmuls (free dim 512) in PSUM f32.
  - BN biases fused into ScalarE activations (per-partition bias).
  - r^T += 3*w^T (.) t^T: product on VectorE; accumulation split between
    TensorE identity-matmuls (2 d-chunks) and GPSIMD adds (2 d-chunks).
  - Single packed output DMA; host unpacks/transposes.
"""

import os
import sys

import numpy as np

sys.path.insert(0, "/opt/trn_rl_repo")

import ml_dtypes

B, NB, DIM, R = 4096, 64, 512, 4
INTER = DIM // R  # 128
EPS = 1e-5
NCORES = 8
BS = B // NCORES  # 512 rows per core
NDC = DIM // 128  # 4 d-chunks
NG = 8            # branches per t DMA group
NGRP = NB // NG   # 8 groups
NSLOT = 5         # t ring-buffer slots

# blob column offsets (bf16 blob, [128, CB])
T_COLS = NDC * NB * BS          # 131072
W_COLS = NB * NDC * INTER       # 32768
OW1 = T_COLS
OW2 = OW1 + W_COLS
OID = OW2 + W_COLS
CB = OID + 128                  # 196736

_CACHE = {}


def _build_nc():
    import concourse.bass as bass
    import concourse.mybir as mybir

    bf16 = mybir.dt.bfloat16
    f32 = mybir.dt.float32
    AF = mybir.ActivationFunctionType
    OP = mybir.AluOpType

    nc = bass.Bass("TRN2", debug=False, target_bir_lowering=False)

    blob_ext = nc.declare_dram_parameter("blob", [128, CB], bf16, isOutput=False)
    bias_ext = nc.declare_dram_parameter("bias", [128, 320], f32, isOutput=False)
    out_ext = nc.declare_dram_parameter("out", [128, NDC * BS], f32, isOutput=True)

    def tcol(g, dc):
        return dc * (NB * BS) + g * (NG * BS)

    R_PE = (0, 1)    # dc accumulated on PE (PSUM, exact)
    R_POOL = (2, 3)  # dc accumulated on GPSIMD (f32 adds in SBUF)

    from contextlib import ExitStack
    ctx = ExitStack()
    with ctx:
        s_w = ctx.enter_context(nc.semaphore("s_w"))
        s_out = ctx.enter_context(nc.semaphore("s_out"))
        s_slot = [ctx.enter_context(nc.semaphore(f"s_slot{i}")) for i in range(NSLOT)]
        s_pe = ctx.enter_context(nc.semaphore("s_pe"))
        s_act = ctx.enter_context(nc.semaphore("s_act"))
        s_dve = ctx.enter_context(nc.semaphore("s_dve"))
        s_pool = ctx.enter_context(nc.semaphore("s_pool"))
        ident_sb = ctx.enter_context(nc.sbuf_tensor("ident_sb", [128, 128], bf16))
        w1_sb = ctx.enter_context(nc.sbuf_tensor("w1_sb", [128, NB, NDC, INTER], bf16))
        w2_sb = ctx.enter_context(nc.sbuf_tensor("w2_sb", [INTER, NB, NDC, 128], bf16))
        b1_sb = ctx.enter_context(nc.sbuf_tensor("b1_sb", [INTER, NB], f32))
        b2_sb = ctx.enter_context(nc.sbuf_tensor("b2_sb", [128, NB, NDC], f32))
        tt_sb = ctx.enter_context(nc.sbuf_tensor("tt_sb", [128, NSLOT, NG, BS], bf16))
        s_sb = ctx.enter_context(nc.sbuf_tensor("s_sb", [128, NDC, BS], bf16))
        h_sb = ctx.enter_context(nc.sbuf_tensor("h_sb", [INTER, 2, BS], bf16))
        w_sb = ctx.enter_context(nc.sbuf_tensor("w_sb", [128, NDC, 2, BS], bf16))
        p_sb = ctx.enter_context(nc.sbuf_tensor("p_sb", [128, NDC, 2, BS], bf16))
        racc_sb = ctx.enter_context(nc.sbuf_tensor("racc_sb", [128, NDC, BS], f32))
        ps_s = [ctx.enter_context(nc.psum_tensor(f"ps_s{dc}", [128, BS], f32)) for dc in range(NDC)]
        ps_y = [ctx.enter_context(nc.psum_tensor(f"ps_y{i}", [128, BS], f32)) for i in range(2)]
        ps_h = [ctx.enter_context(nc.psum_tensor(f"ps_h{i}", [INTER, BS], f32)) for i in range(2)]
        y_bank = {0: ps_y[0], 1: ps_y[1], 2: ps_s[2], 3: ps_s[3]}
        r_bank = {0: ps_s[0], 1: ps_s[1]}

        # ---------- static schedule ----------
        n_tdma = 2 * NGRP * NDC
        tslot_done = [16 * (k // NSLOT + 1) for k in range(n_tdma)]

        def tk(phase, g, dc):
            return phase * NGRP * NDC + g * NDC + dc

        pe_groupA_done = [None] * (NGRP * NDC)
        h_ready = [None] * NB
        y_ready = [[None] * NDC for _ in range(NB)]
        racc_done_pe = [[None] * NDC for _ in range(NB)]
        relu_done = [None] * NB
        sig_done = [[None] * NDC for _ in range(NB)]
        scopy_done = [None] * NDC
        stt_done = [[None] * NDC for _ in range(NB)]
        pool_done = [[None] * NDC for _ in range(NB)]
        rcopy_done = [None] * NDC

        pe_i = 0
        for k in range(NGRP * NDC):
            pe_i += 1
            pe_groupA_done[k] = pe_i
        pe_i += 1
        h_ready[0] = pe_i
        for n in range(NB):
            if n + 1 < NB:
                pe_i += 1
                h_ready[n + 1] = pe_i
            for dc in range(NDC):
                pe_i += 1
                y_ready[n][dc] = pe_i
            if n > 0:
                for dc in R_PE:
                    pe_i += 1
                    racc_done_pe[n - 1][dc] = pe_i
        for dc in R_PE:
            pe_i += 1
            racc_done_pe[NB - 1][dc] = pe_i

        act_i = 1
        relu_done[0] = 1
        for n in range(NB):
            if n + 1 < NB:
                act_i += 1
                relu_done[n + 1] = act_i
            for dc in range(NDC):
                act_i += 1
                sig_done[n][dc] = act_i

        dve_i = 0
        for dc in range(NDC):
            dve_i += 1
            scopy_done[dc] = dve_i
        for n in range(NB):
            for dc in range(NDC):
                dve_i += 1
                stt_done[n][dc] = dve_i
        for dc in R_PE:
            dve_i += 1
            rcopy_done[dc] = dve_i

        pool_i = 0
        for n in range(NB):
            for dc in R_POOL:
                pool_i += 1
                pool_done[n][dc] = pool_i

        # weight-group availability: s_w counts (ident, b1, b2, w1g0, w2g0,
        # w1g1, w2g1, then per-group pairs issued inside the pool r-loop)
        def wg_done(g):
            return 16 * (3 + 2 * (g + 1))

        # t-chunk DMA issue helpers (phase A chunks alternate SP/Act)
        def issue_phaseA(eng, parity):
            for k in range(NGRP * NDC):
                if k % 2 != parity:
                    continue
                g, dc = divmod(k, NDC)
                slot = k % NSLOT
                if k >= NSLOT:
                    eng.wait_ge(s_pe, pe_groupA_done[k - NSLOT])
                c0 = tcol(g, dc)
                eng.dma_start(
                    out=tt_sb[:, slot, :, :],
                    in_=blob_ext[:, c0:c0 + NG * BS],
                ).then_inc(s_slot[slot], 16)

        with nc.Block() as block:

            # ================= SP: phase-A (even chunks) + all phase-B t + out =================
            @block.sync
            def _(sp):
                issue_phaseA(sp, 0)
                for g in range(NGRP):
                    for dc in range(NDC):
                        k = tk(1, g, dc)
                        slot = k % NSLOT
                        pk = k - NSLOT
                        if pk < NGRP * NDC:
                            sp.wait_ge(s_pe, pe_groupA_done[pk])
                        else:
                            m = pk - NGRP * NDC
                            pg, pdc = divmod(m, NDC)
                            pn = pg * NG + NG - 1
                            sp.wait_ge(s_dve, stt_done[pn][pdc])
                        c0 = tcol(g, dc)
                        sp.dma_start(
                            out=tt_sb[:, slot, :, :],
                            in_=blob_ext[:, c0:c0 + NG * BS],
                        ).then_inc(s_slot[slot], 16)
                for dc in R_PE:
                    sp.wait_ge(s_dve, rcopy_done[dc])
                for dc in R_POOL:
                    sp.wait_ge(s_pool, pool_done[NB - 1][dc])
                sp.dma_start(out=out_ext[:, :], in_=racc_sb[:, :, :]).then_inc(s_out, 16)
                sp.wait_ge(s_out, 16)

            # ================= PE =================
            @block.tensor
            def _(pe):
                pe.wait_ge(s_w, 16 * 3)  # ident+biases
                for g in range(NGRP):
                    for dc in range(NDC):
                        k = tk(0, g, dc)
                        slot = k % NSLOT
                        pe.wait_ge(s_slot[slot], tslot_done[k])
                        for j in range(NG):
                            mm = pe.matmul(
                                ps_s[dc][:, :], lhsT=ident_sb[:, :],
                                rhs=tt_sb[:, slot, j, :],
                                start=(g == 0 and j == 0), stop=(g == NGRP - 1 and j == NG - 1),
                            )
                        mm.then_inc(s_pe, 1)
                # prologue: G1(0)
                pe.wait_ge(s_dve, scopy_done[NDC - 1])
                pe.wait_ge(s_w, wg_done(0))
                for dc in range(NDC):
                    mm = pe.matmul(
                        ps_h[0][:, :], lhsT=w1_sb[:, 0, dc, :], rhs=s_sb[:, dc, :],
                        start=(dc == 0), stop=(dc == NDC - 1),
                    )
                mm.then_inc(s_pe, 1)
                for n in range(NB):
                    # G1(n+1) hoisted: h always ready one branch ahead
                    if n + 1 < NB:
                        if (n + 1) % NG == 0:
                            pe.wait_ge(s_w, wg_done((n + 1) // NG))
                        for dc in range(NDC):
                            mm = pe.matmul(
                                ps_h[(n + 1) % 2][:, :], lhsT=w1_sb[:, n + 1, dc, :], rhs=s_sb[:, dc, :],
                                start=(dc == 0), stop=(dc == NDC - 1),
                            )
                        mm.then_inc(s_pe, 1)
                    pe.wait_ge(s_act, relu_done[n])
                    for dc in range(NDC):
                        if n > 0:
                            pe.wait_ge(s_act, sig_done[n - 1][dc])
                        elif dc >= 2:
                            pe.wait_ge(s_dve, scopy_done[dc])
                        pe.matmul(
                            y_bank[dc][:, :], lhsT=w2_sb[:, n, dc, :], rhs=h_sb[:, n % 2, :],
                            start=True, stop=True,
                        ).then_inc(s_pe, 1)
                    if n > 0:
                        for dc in R_PE:
                            pe.wait_ge(s_dve, stt_done[n - 1][dc])
                            pe.matmul(
                                r_bank[dc][:, :], lhsT=ident_sb[:, :], rhs=p_sb[:, dc, (n - 1) % 2, :],
                                start=(n - 1 == 0), stop=False,
                            ).then_inc(s_pe, 1)
                for dc in R_PE:
                    pe.wait_ge(s_dve, stt_done[NB - 1][dc])
                    pe.matmul(
                        r_bank[dc][:, :], lhsT=ident_sb[:, :], rhs=p_sb[:, dc, (NB - 1) % 2, :],
                        start=False, stop=True,
                    ).then_inc(s_pe, 1)

            # ================= ACT: phase-A odd t chunks, then relu/sigmoid =================
            @block.scalar
            def _(act):
                issue_phaseA(act, 1)
                act.wait_ge(s_w, 16 * 3)
                act.wait_ge(s_pe, h_ready[0])
                act.activation(
                    h_sb[:, 0, :], ps_h[0][:, :], AF.Relu,
                    bias=b1_sb[:, 0:1], scale=1.0,
                ).then_inc(s_act, 1)
                for n in range(NB):
                    if n + 1 < NB:
                        act.wait_ge(s_pe, h_ready[n + 1])
                        if n >= 1:
                            act.wait_ge(s_pe, y_ready[n - 1][NDC - 1])  # h WAR
                        act.activation(
                            h_sb[:, (n + 1) % 2, :], ps_h[(n + 1) % 2][:, :], AF.Relu,
                            bias=b1_sb[:, n + 1:n + 2], scale=1.0,
                        ).then_inc(s_act, 1)
                    if n >= 2:
                        act.wait_ge(s_dve, stt_done[n - 2][NDC - 1])
                    for dc in range(NDC):
                        act.wait_ge(s_pe, y_ready[n][dc])
                        act.activation(
                            w_sb[:, dc, n % 2, :], y_bank[dc][:, :], AF.Sigmoid,
                            bias=b2_sb[:, n, dc:dc + 1], scale=1.0,
                        ).then_inc(s_act, 1)

            # ================= DVE =================
            @block.vector
            def _(dve):
                dve.wait_ge(s_pe, pe_groupA_done[NGRP * NDC - 1])
                for dc in range(NDC):
                    dve.tensor_copy(s_sb[:, dc, :], ps_s[dc][:, :]).then_inc(s_dve, 1)
                for n in range(NB):
                    g, j = divmod(n, NG)
                    for dc in range(NDC):
                        if j == 0:
                            k = tk(1, g, dc)
                            dve.wait_ge(s_slot[k % NSLOT], tslot_done[k])
                        dve.wait_ge(s_act, sig_done[n][dc])
                        if n >= 2:
                            if dc in R_PE:
                                dve.wait_ge(s_pe, racc_done_pe[n - 2][dc])
                            else:
                                dve.wait_ge(s_pool, pool_done[n - 2][dc])
                        slot = tk(1, g, dc) % NSLOT
                        dve.scalar_tensor_tensor(
                            p_sb[:, dc, n % 2, :], in0=w_sb[:, dc, n % 2, :], scalar=3.0,
                            in1=tt_sb[:, slot, j, :], op0=OP.mult, op1=OP.mult,
                        ).then_inc(s_dve, 1)
                for dc in R_PE:
                    dve.wait_ge(s_pe, racc_done_pe[NB - 1][dc])
                    dve.tensor_copy(racc_sb[:, dc, :], r_bank[dc][:, :]).then_inc(s_dve, 1)

            # ================= GPSIMD: weight DMAs (paced) + r accumulation =================
            @block.gpsimd
            def _(pool):
                pool.dma_start(out=ident_sb[:, :], in_=blob_ext[:, OID:OID + 128]).then_inc(s_w, 16)
                pool.dma_start(out=b1_sb[:, :], in_=bias_ext[:, 0:64]).then_inc(s_w, 16)
                pool.dma_start(out=b2_sb[:, :, :], in_=bias_ext[:, 64:320]).then_inc(s_w, 16)

                def wdma(g):
                    a, b = g * NG, (g + 1) * NG
                    pool.dma_start(
                        out=w1_sb[:, a:b, :, :],
                        in_=blob_ext[:, OW1 + a * NDC * INTER:OW1 + b * NDC * INTER],
                    ).then_inc(s_w, 16)
                    pool.dma_start(
                        out=w2_sb[:, a:b, :, :],
                        in_=blob_ext[:, OW2 + a * NDC * INTER:OW2 + b * NDC * INTER],
                    ).then_inc(s_w, 16)

                wdma(0)
                wdma(1)
                for n in range(NB):
                    # pace weight group g = n//NG + 2 at each group boundary
                    if n % NG == 0 and n // NG + 2 < NGRP:
                        wdma(n // NG + 2)
                    for dc in R_POOL:
                        pool.wait_ge(s_dve, stt_done[n][dc])
                        if n == 0:
                            pool.tensor_copy(
                                racc_sb[:, dc, :], p_sb[:, dc, 0, :]
                            ).then_inc(s_pool, 1)
                        else:
                            pool.tensor_add(
                                racc_sb[:, dc, :], racc_sb[:, dc, :], p_sb[:, dc, n % 2, :]
                            ).then_inc(s_pool, 1)
                    pool.drain()

    return nc


def _prep(inputs):
    t = inputs["t"]
    W1, b1, g1, beta1, m1, v1 = (
        inputs["W1"], inputs["b1"], inputs["g1"], inputs["beta1"],
        inputs["m1"], inputs["v1"],
    )
    W2, b2, g2, beta2, m2, v2 = (
        inputs["W2"], inputs["b2"], inputs["g2"], inputs["beta2"],
        inputs["m2"], inputs["v2"],
    )
    a1 = g1 / np.sqrt(v1 + EPS)  # (NB, I)
    W1f = W1 * a1[:, :, None]  # (NB, I, D)
    b1f = (b1 - m1) * a1 + beta1  # (NB, I)
    a2 = g2 / np.sqrt(v2 + EPS)  # (NB, D)
    W2f = W2 * a2[:, :, None]  # (NB, D, I)
    b2f = (b2 - m2) * a2 + beta2  # (NB, D)

    bf16 = ml_dtypes.bfloat16
    # w1t[p, n, dc, i] = W1f[n, i, dc*128+p]
    w1t = np.ascontiguousarray(
        W1f.reshape(NB, INTER, NDC, 128).transpose(3, 0, 2, 1)
    ).astype(bf16).reshape(128, W_COLS)
    # w2t[i, n, dc, dd] = W2f[n, dc*128+dd, i]
    w2t = np.ascontiguousarray(
        W2f.reshape(NB, NDC, 128, INTER).transpose(3, 0, 1, 2)
    ).astype(bf16).reshape(128, W_COLS)
    ident = np.eye(128, dtype=bf16)
    # t block: [p, dc, n, b] = t[b, n, dc*128+p]
    t_bf = t.astype(bf16)  # (B, NB, D)
    t_cols = np.ascontiguousarray(
        t_bf.transpose(2, 1, 0).reshape(NDC, 128, NB, B).transpose(1, 0, 2, 3)
    )  # (128, NDC, NB, B)

    # bias blob f32 [128, 320]: b1 (64) | b2 (256)
    bias = np.empty((128, 320), np.float32)
    bias[:, 0:64] = b1f.T
    bias[:, 64:320] = b2f.reshape(NB, NDC, 128).transpose(2, 0, 1).reshape(128, 256)

    blobs = []
    for c in range(NCORES):
        blob = np.empty((128, CB), bf16)
        blob[:, 0:T_COLS] = t_cols[:, :, :, c * BS:(c + 1) * BS].reshape(128, T_COLS)
        blob[:, OW1:OW1 + W_COLS] = w1t
        blob[:, OW2:OW2 + W_COLS] = w2t
        blob[:, OID:OID + 128] = ident
        blobs.append(blob)
    return blobs, bias


def kernel(**inputs):
    from concourse.bass_utils import run_bass_kernel_spmd

    blobs, bias = _prep(inputs)

    if "nc" not in _CACHE:
        _CACHE["nc"] = _build_nc()
    nc = _CACHE["nc"]

    in_maps = []
    for c in range(NCORES):
        in_maps.append({"blob": blobs[c], "bias": bias})
    res = run_bass_kernel_spmd(nc, in_maps, core_ids=list(range(NCORES)))
    outs = []
    for c in range(NCORES):
        o = res.results[c]["out"]  # (128, NDC*BS)
        outs.append(o.reshape(128, NDC, BS).transpose(2, 1, 0).reshape(BS, DIM))
    return np.concatenate(outs, axis=0).astype(np.float32)


if __name__ == "__main__":
    rng = np.random.default_rng(0)
    fake = {
        "t": rng.standard_normal((B, NB, DIM), dtype=np.float32),
        "W1": rng.standard_normal((NB, INTER, DIM), dtype=np.float32) * 0.02,
        "b1": rng.standard_normal((NB, INTER), dtype=np.float32) * 0.02,
        "g1": 1 + 0.1 * rng.standard_normal((NB, INTER), dtype=np.float32),
        "beta1": 0.1 * rng.standard_normal((NB, INTER), dtype=np.float32),
        "m1": 0.1 * rng.standard_normal((NB, INTER), dtype=np.float32),
        "v1": rng.uniform(0.5, 1.5, (NB, INTER)).astype(np.float32),
        "W2": rng.standard_normal((NB, DIM, INTER), dtype=np.float32) * 0.02,
        "b2": rng.standard_normal((NB, DIM), dtype=np.float32) * 0.02,
        "g2": 1 + 0.1 * rng.standard_normal((NB, DIM), dtype=np.float32),
        "beta2": 0.1 * rng.standard_normal((NB, DIM), dtype=np.float32),
        "m2": 0.1 * rng.standard_normal((NB, DIM), dtype=np.float32),
        "v2": rng.uniform(0.5, 1.5, (NB, DIM)).astype(np.float32),
    }
    out = kernel(**fake)
    print("kernel ran, out shape", out.shape, out.dtype)
    # quick numpy check of the math
    s = fake["t"].sum(axis=1)
    h = np.einsum('nid,bd->bni', fake["W1"], s) + fake["b1"]
    h = (h - fake["m1"]) / np.sqrt(fake["v1"] + EPS) * fake["g1"] + fake["beta1"]
    h = np.maximum(h, 0)
    y = np.einsum('ndi,bni->bnd', fake["W2"], h) + fake["b2"]
    y = (y - fake["m2"]) / np.sqrt(fake["v2"] + EPS) * fake["g2"] + fake["beta2"]
    w = 1 / (1 + np.exp(-y))
    ref = (w * fake["t"]).sum(axis=1) * 3.0
    err = np.linalg.norm(out - ref) / np.linalg.norm(ref)
    print("rel err vs numpy:", err)
